# revision 2
# baseline (speedup 1.0000x reference)
"""BitLinear (ternary-weight linear) Trainium2 kernel — fp8 DoubleRow version.

Computes  Y = x @ ternarize(W).T + bias  where
  ternarize(W) = {-1, 0, +1} via threshold t = 0.05 * max(mean(|W|), 1e-6)
with x: [8192, 4096] f32, W: [16384, 4096] f32, bias: [16384] f32.

Strategy: column-parallel tensor parallelism over 8 NeuronCores, with the
matmul in fp8e4m3 DoubleRow mode (PE packs 2 fp8 weights per cell, 2
MACs/cell/cycle — 2x bf16 FLOP rate):
  - x is quantized host-side to e4m3 ("hi") plus e4m3 residuals ("lo") for
    the first NLO 256-wide k-chunks; W stays exact ({-1,0,+1} is
    e4m3-representable).  Measured on the real data: rel err 0.0195 with
    hi only, 0.0189 at NLO=2, 0.0174 at NLO=3 (gate 2e-2).
  - On-device: per-core |W| partial sum -> AllReduce(8) -> global threshold
    -> ternarize shard to fp8 pairs layout in a DRAM scratch -> DoubleRow
    matmuls, W-stationary ([128,2,128] fp8 tiles via LDWEIGHTS), moving
    x slabs [128,2,4096], psum [128o,512t] f32 accumulated over 16 (+NLO)
    chunks -> +bias -> f32 y^T shard [2048, 8192].
  - The lo matmuls reuse the already-loaded stationary W tile (no extra
    LDWEIGHTS).  Host assembles y = vstack(shards).T (pure view).
"""

import numpy as np
import ml_dtypes

import concourse.bass as bass
import concourse.bacc as bacc
import concourse.tile as tile
import concourse.mybir as mybir
import concourse.bass_isa as bass_isa
from concourse import bass_utils

F32 = mybir.dt.float32
BF16 = mybir.dt.bfloat16
FP8 = mybir.dt.float8e4
NP8 = ml_dtypes.float8_e4m3

N_CORES = 8
TOKENS = 8192
K_FEAT = 4096
OUT_FEAT = 16384

P = 128
NB = 512          # moving tokens per matmul (one psum bank of f32)
G = 8             # token-chunks per group (psum banks used)
NLO = 2           # lo-corrected 256-wide k-chunks

THRESHOLD = 0.05
EPS = 1e-6


def _ldw_sig(inst):
    a = inst.ins[0]
    return (a.memref, a.offset, str(a.ap), str(a.dtype),
            str(inst.perf_mode), str(inst.is_transpose), str(inst.tile_position))


def _dedupe_ldweights(nc):
    """Remove PE LDWEIGHTS that reload the stationary operand already in the
    array (identical AP, only MATMULs in between). Tile lowers every matmul to
    an LDWEIGHTS+MATMUL pair; with G matmuls per stationary tile the reload
    wastes PE cycles. Deleted LDW waits move onto the next PE instruction."""
    n_removed = 0
    for bb in nc.main_func.blocks:
        insts = bb.instructions
        last_sig = None
        pending_waits = []
        keep = []
        for inst in insts:
            if inst.engine != mybir.EngineType.PE:
                keep.append(inst)
                continue
            if isinstance(inst, mybir.InstLdweights):
                si = inst.sync_info
                has_updates = si is not None and len(si.on_update) > 0
                sig = _ldw_sig(inst)
                if sig == last_sig and not has_updates and not inst.ins[0].regs_read():
                    if si is not None and len(si.on_wait) > 0:
                        pending_waits.extend(si.on_wait)
                    n_removed += 1
                    continue
                last_sig = sig
            elif isinstance(inst, mybir.InstMatmult):
                pass  # matmuls don't disturb the loaded weights
            else:
                last_sig = None
            if pending_waits:
                si = inst.sync_info
                if si is None:
                    inst.sync_info = mybir.SyncInfo(
                        on_wait=list(pending_waits), on_update=[]
                    )
                else:
                    si.on_wait = list(pending_waits) + list(si.on_wait)
                pending_waits = []
            keep.append(inst)
        assert not pending_waits, "trailing LDW waits with no PE successor"
        if len(keep) != len(insts):
            while len(insts):
                insts.pop()
            for inst in keep:
                insts.append(inst)
    return n_removed


def build_kernel(tokens=TOKENS, k_feat=K_FEAT, out_feat=OUT_FEAT, n_cores=N_CORES,
                 use_collective=True, compile=True, nlo=NLO, g=G, cache_salt=0,
                 dedupe_ldw=True):
    """Build + compile the per-core Bass program (SPMD, symmetric)."""
    o_shard = out_feat // n_cores           # 2048
    kc = k_feat // 256                      # 16 k-chunks (DoubleRow pairs)
    og_tiles = o_shard // P                 # 16 output tiles
    k_tiles = k_feat // P                   # 32 (phase A/B granularity)
    tg_n = tokens // (g * NB)               # token groups

    nc = bacc.Bacc("TRN2", target_bir_lowering=False, debug=False, num_devices=n_cores)

    # xq[tg, c, p, s, t] = e4m3(x)[tg*g*NB + t, c*256 + s*128 + p]
    xq_d = nc.dram_tensor("xq", [tg_n, kc, P, 2, g * NB], FP8, kind="ExternalInput")
    xlo_d = (nc.dram_tensor("xlo", [tg_n, nlo, P, 2, g * NB], FP8,
                            kind="ExternalInput") if nlo else None)
    # wt[k, o] = W[o_global, k] for this core's o-shard (f32)
    wt_d = nc.dram_tensor("wt", [k_feat, o_shard], F32, kind="ExternalInput")
    bias_d = nc.dram_tensor("bias", [o_shard, 1], F32, kind="ExternalInput")
    # y^T shard
    y_d = nc.dram_tensor("y", [o_shard, tokens], F32, kind="ExternalOutput")

    with tile.TileContext(nc) as tc:
        with (
            tc.tile_pool(name="singles", bufs=1) as singles,
            tc.tile_pool(name="wstage", bufs=2) as wstage,
            tc.tile_pool(name="b01", bufs=2) as b01_pool,
            tc.tile_pool(name="wq8", bufs=2) as wq8_pool,
            tc.tile_pool(name="wsl", bufs=2) as wsl_pool,
            tc.tile_pool(name="xs", bufs=1) as xs_pool,
            tc.tile_pool(name="out", bufs=3) as out_pool,
            tc.tile_pool(name="psum", bufs=1, space="PSUM") as psum_pool,
            tc.tile_pool(name="dram", bufs=1, space="DRAM") as dram,
        ):
            # ---------- Phase A: global scale = mean(|W|) ----------
            acc = singles.tile([P, k_tiles], F32)
            for i in range(k_tiles):
                w_i = wstage.tile([P, o_shard], F32, name="wstage")
                nc.sync.dma_start(w_i[:], wt_d[i * P:(i + 1) * P, :])
                nc.vector.tensor_reduce(
                    acc[:, i:i + 1], w_i[:],
                    axis=mybir.AxisListType.X, op=mybir.AluOpType.add,
                    apply_absolute_value=True,
                )
            colsum = singles.tile([P, 1], F32)
            nc.vector.tensor_reduce(
                colsum[:], acc[:], axis=mybir.AxisListType.X, op=mybir.AluOpType.add
            )
            # partition sum via PE (idle here): [1,1] = colsum.T @ ones
            ones = singles.tile([P, 1], F32)
            nc.vector.memset(ones[:], 1.0)
            ps_sc = psum_pool.tile([P, NB], F32, name="ps_0")
            nc.tensor.matmul(ps_sc[0:1, 0:1], colsum[:], ones[:])
            ssum8 = singles.tile([1, 8], F32)
            nc.vector.memset(ssum8[:], 0.0)
            for _ in range(cache_salt):  # perturb BIR hash for A/B compiles
                nc.vector.memset(ssum8[:, 7:8], 0.0)
            nc.vector.tensor_copy(ssum8[:, 0:1], ps_sc[0:1, 0:1])
            in_b = dram.tile([1, 8], F32)
            out_b = dram.tile([1, 8], F32)
            nc.gpsimd.dma_start(in_b[:], ssum8[:])
            if use_collective:
                nc.gpsimd.collective_compute(
                    "AllReduce",
                    mybir.AluOpType.add,
                    replica_groups=[list(range(n_cores))],
                    ins=[in_b.opt()],
                    outs=[out_b.opt()],
                )
            else:  # single-core / TimelineSim variant
                nc.gpsimd.dma_start(out_b[:], in_b[:])
            gsum = singles.tile([1, 8], F32)
            nc.gpsimd.dma_start(gsum[:], out_b[:])

            # thr = 0.05 * max(gsum/(out*k), eps); also need -thr
            scale_p0 = singles.tile([1, 1], F32)
            nc.vector.tensor_scalar(
                scale_p0[:], gsum[0:1, 0:1],
                1.0 / (out_feat * k_feat), EPS,
                op0=mybir.AluOpType.mult, op1=mybir.AluOpType.max,
            )
            thr_p0 = singles.tile([1, 1], F32)
            nthr_p0 = singles.tile([1, 1], F32)
            nc.vector.tensor_scalar_mul(thr_p0[:], scale_p0[:], THRESHOLD)
            nc.vector.tensor_scalar_mul(nthr_p0[:], scale_p0[:], -THRESHOLD)
            thr = singles.tile([P, 1], F32)
            nthr = singles.tile([P, 1], F32)
            nc.gpsimd.partition_broadcast(thr[:], thr_p0[:])
            nc.gpsimd.partition_broadcast(nthr[:], nthr_p0[:])

            # bias: [o_shard, 1] -> SBUF [128, og] columns
            bias_sb = singles.tile([P, og_tiles], F32)
            for og in range(og_tiles):
                nc.sync.dma_start(
                    bias_sb[:, og:og + 1], bias_d[og * P:(og + 1) * P, :]
                )

            # ---------- Phase B: ternarize shard -> fp8 pairs in DRAM ----------
            # wq_dram[c, p, s, o] = tern(W)[o, c*256 + s*128 + p]
            wq_dram = dram.tile([kc, P, 2, o_shard], FP8)
            for i in range(k_tiles):
                w_i = wstage.tile([P, o_shard], F32, name="wstage")
                nc.sync.dma_start(w_i[:], wt_d[i * P:(i + 1) * P, :])
                b01 = b01_pool.tile([P, o_shard], BF16, name="b01")
                wq_i = wq8_pool.tile([P, o_shard], FP8, name="wq8")
                for q in range(o_shard // NB):
                    sl = slice(q * NB, (q + 1) * NB)
                    nc.vector.tensor_scalar(
                        b01[:, sl], w_i[:, sl], nthr[:], None,
                        op0=mybir.AluOpType.is_lt,
                    )
                    nc.vector.scalar_tensor_tensor(
                        wq_i[:, sl], w_i[:, sl], thr[:], b01[:, sl],
                        op0=mybir.AluOpType.is_gt, op1=mybir.AluOpType.subtract,
                    )
                nc.sync.dma_start(wq_dram[i // 2, :, i % 2, :], wq_i[:])

            # ---------- Phase C: DoubleRow matmuls + bias ----------
            for tg in range(tg_n):
                xs = []
                for c in range(kc):
                    x_c = xs_pool.tile([P, 2, g * NB], FP8, name=f"xs_{c}")
                    nc.sync.dma_start(x_c[:], xq_d[tg, c])
                    xs.append(x_c)
                xlo = []
                for c in range(nlo):
                    xl_c = xs_pool.tile([P, 2, g * NB], FP8, name=f"xlo_{c}")
                    nc.sync.dma_start(xl_c[:], xlo_d[tg, c])
                    xlo.append(xl_c)
                for og in range(og_tiles):
                    wsl = []
                    for c in range(kc):
                        w_c = wsl_pool.tile([P, 2, P], FP8, name=f"wsl_{c}")
                        nc.sync.dma_start(
                            w_c[:], wq_dram[c, :, :, og * P:(og + 1) * P]
                        )
                        wsl.append(w_c)
                    ps = [psum_pool.tile([P, NB], F32, name=f"ps_{t}")
                          for t in range(g)]
                    for c in range(kc):
                        for t in range(g):
                            nc.tensor.matmul(
                                ps[t][:], wsl[c][:],
                                xs[c][:, :, t * NB:(t + 1) * NB],
                                start=(c == 0), stop=(c == kc - 1),
                                perf_mode=mybir.MatmulPerfMode.DoubleRow,
                            )
                        if c < nlo:
                            for t in range(g):
                                nc.tensor.matmul(
                                    ps[t][:], wsl[c][:],
                                    xlo[c][:, :, t * NB:(t + 1) * NB],
                                    start=False, stop=False,
                                    perf_mode=mybir.MatmulPerfMode.DoubleRow,
                                )
                    for t in range(g):
                        ot = out_pool.tile([P, NB], F32, name="ot")
                        nc.vector.tensor_scalar(
                            ot[:], ps[t][:], bias_sb[:, og:og + 1], None,
                            op0=mybir.AluOpType.add,
                        )
                        nc.sync.dma_start(
                            y_d[og * P:(og + 1) * P,
                                (tg * g + t) * NB:(tg * g + t + 1) * NB],
                            ot[:],
                        )

    if dedupe_ldw:
        n = _dedupe_ldweights(nc)
        import logging
        logging.getLogger(__name__).info("dedupe_ldweights removed %d", n)
    if compile:
        nc.compile()
    return nc


def make_in_maps(x, weight, bias, tokens=TOKENS, k_feat=K_FEAT, out_feat=OUT_FEAT,
                 n_cores=N_CORES, nlo=NLO, g=G):
    """Host-side marshalling: quantize x, shard + relayout per core."""
    o_shard = out_feat // n_cores
    kc = k_feat // 256
    tg_n = tokens // (g * NB)
    tl = g * NB

    x = np.ascontiguousarray(x, dtype=np.float32)
    x8 = x.astype(NP8)
    # xq[tg, c, p, s, t] = x8[tg*tl + t, c*256 + s*128 + p]
    def relayout(a8):
        return np.ascontiguousarray(
            a8.reshape(tg_n, tl, kc, 2, P).transpose(0, 2, 4, 3, 1)
        )
    xq = relayout(x8)
    in_maps = []
    xlo = None
    if nlo:
        xf = x[:, :nlo * 256] - x8[:, :nlo * 256].astype(np.float32)
        xlo8 = xf.astype(NP8)
        # [tg, c, p, s, t] over the first nlo chunks
        xlo = np.ascontiguousarray(
            xlo8.reshape(tg_n, tl, nlo, 2, P).transpose(0, 2, 4, 3, 1)
        )
    for c in range(n_cores):
        wt_c = np.ascontiguousarray(weight[c * o_shard:(c + 1) * o_shard, :].T)
        bias_c = np.ascontiguousarray(
            bias[c * o_shard:(c + 1) * o_shard]
        ).reshape(o_shard, 1)
        m = {"xq": xq, "wt": wt_c, "bias": bias_c}
        if nlo:
            m["xlo"] = xlo
        in_maps.append(m)
    return in_maps


_CACHED_NC = None


def kernel(x: np.ndarray, weight: np.ndarray, bias: np.ndarray) -> np.ndarray:
    global _CACHED_NC
    if _CACHED_NC is None:
        _CACHED_NC = build_kernel()
    nc = _CACHED_NC
    in_maps = make_in_maps(x, weight, bias)
    res = bass_utils.run_bass_kernel_spmd(nc, in_maps, core_ids=list(range(N_CORES)))
    y = np.vstack([res.results[c]["y"] for c in range(N_CORES)]).T
    assert y.shape == (TOKENS, OUT_FEAT) and y.dtype == np.float32
    return y
